# revision 10
# baseline (speedup 1.0000x reference)
"""HGT (nn_HGT_56152402427945) Trainium2 Bass kernel, 8 cores, edge/dst-sharded.

- Nodes sharded: papers 8x12500, authors 8x6250; each core owns edges whose dst
  is in its slice, so segment-softmax and aggregation are core-local.
- Hidden states are AllGathered into a rank-major table x_all [150000,128].
- Per-relation fused weights (W~k = Wk @ blockdiag(a_rel * p/sqrt(D)), etc.)
  turn the per-edge transform into x[src] @ W~ - one 512B gather per edge row.
- Edges packed by dst into stride-S tiles (tile t: dst in [S t, S(t+1)),
  lane = dst - S t, <=128 edges/tile, S chosen from the data). All addressing
  is affine in t -> tight For_i hardware loops, dense U tables, dense q loads.
- Per tile: gather x_src; one-hot(lane) matmuls expand q rows to edges and
  aggregate exp(score)-weighted messages + exp-sums (den).
- Epilogue: U_cites+U_writes (papers) / U_rev (authors), normalize, gelu,
  a-lin, sigmoid-skip mix; AllGather the new x. Final out_lin on papers.
"""

import numpy as np

NP_, NA = 100000, 50000
DIN, HID, H, D, OUT = 128, 128, 4, 32, 10
LAYERS = 2
NC = 8
PSL, ASL = NP_ // NC, NA // NC  # 12500 / 6250
SL = PSL + ASL                  # 18750
SQRT_D = np.float32(np.sqrt(D))
P = 128
UN = 8  # unroll for hardware loops

EDGE_TYPES = [("paper", "cites", "paper"), ("author", "writes", "paper"),
              ("paper", "rev_writes", "author")]


# ------------------------------------------------------------ host preprocess
def _pick_stride(dst_all, n_dst):
    for S in (48, 44, 40, 36, 32, 28, 24, 20, 16, 12, 8):
        ok = True
        for d in dst_all:
            cnt = np.bincount(d // S, minlength=(n_dst + S - 1) // S)
            if cnt.max(initial=0) > 128:
                ok = False
                break
        if ok:
            return S
    raise ValueError("no stride fits")


def _build_edge_tiles(src_rows, dst_local, S, n_tiles):
    t_of = dst_local // S
    order = np.argsort(t_of, kind="stable")
    gidx = np.zeros((P, n_tiles), np.int32)
    ln = np.full((P, n_tiles), 999.0, np.float32)
    ts = t_of[order]
    ssrc = src_rows[order]
    slane = (dst_local - t_of * S)[order]
    start = np.searchsorted(ts, np.arange(n_tiles), side="left")
    end = np.searchsorted(ts, np.arange(n_tiles), side="right")
    for t in range(n_tiles):
        a, b = start[t], end[t]
        k = b - a
        assert k <= P, f"tile {t} overflow {k}"
        gidx[:k, t] = ssrc[a:b]
        ln[:k, t] = slane[a:b].astype(np.float32)
    return gidx, ln


def _fuse_rel_weights(lp, r, s_type):
    Wk = np.asarray(lp["k"][s_type][0], np.float64)
    bk = np.asarray(lp["k"][s_type][1], np.float64)
    Wv = np.asarray(lp["v"][s_type][0], np.float64)
    bv = np.asarray(lp["v"][s_type][1], np.float64)
    a_rel = np.asarray(lp["rel"][r]["a_rel"], np.float64)
    m_rel = np.asarray(lp["rel"][r]["m_rel"], np.float64)
    p_rel = np.asarray(lp["rel"][r]["p_rel"], np.float64)
    Ak = np.zeros((HID, HID))
    Am = np.zeros((HID, HID))
    for h in range(H):
        Ak[h * D:(h + 1) * D, h * D:(h + 1) * D] = a_rel[h] * (p_rel[h] / float(SQRT_D))
        Am[h * D:(h + 1) * D, h * D:(h + 1) * D] = m_rel[h]
    return ((Wk @ Ak).astype(np.float32), (bk @ Ak).astype(np.float32),
            (Wv @ Am).astype(np.float32), (bv @ Am).astype(np.float32))


def _prep(inputs):
    params = inputs["params"]
    percore = [dict() for _ in range(NC)]
    xp = np.asarray(inputs["x_paper"], np.float32)
    xa = np.asarray(inputs["x_author"], np.float32)
    for c in range(NC):
        percore[c]["xin_p"] = xp[c * PSL:(c + 1) * PSL]
        percore[c]["xin_a"] = xa[c * ASL:(c + 1) * ASL]

    rel_info = []
    for (s, r, d) in EDGE_TYPES:
        src = np.asarray(inputs[f"ei_{r}_src"]).astype(np.int64)
        dst = np.asarray(inputs[f"ei_{r}_dst"]).astype(np.int64)
        dsl = ASL if d == "author" else PSL
        core_of = dst // dsl
        if s == "author":
            srow = (src // ASL) * SL + PSL + (src % ASL)
        else:
            srow = (src // PSL) * SL + (src % PSL)
        dst_locals, srows = [], []
        for c in range(NC):
            m = core_of == c
            dst_locals.append((dst[m] - c * dsl).astype(np.int64))
            srows.append(srow[m].astype(np.int32))
        S = _pick_stride(dst_locals, dsl)
        n_tiles = (dsl + S - 1) // S
        n_tiles = ((n_tiles + UN - 1) // UN) * UN
        for c in range(NC):
            g, l = _build_edge_tiles(srows[c], dst_locals[c], S, n_tiles)
            percore[c][f"eidx_{r}"] = g
            percore[c][f"lane_{r}"] = l
        rel_info.append(dict(r=r, s=s, d=d, S=S, n_tiles=n_tiles, n_dst=dsl))

    w = {}
    w["Win_p"] = np.asarray(params["in_lin"]["paper"][0], np.float32)
    w["bin_p"] = np.asarray(params["in_lin"]["paper"][1], np.float32)
    w["Win_a"] = np.asarray(params["in_lin"]["author"][0], np.float32)
    w["bin_a"] = np.asarray(params["in_lin"]["author"][1], np.float32)
    w["Wout"] = np.asarray(params["out_lin"][0], np.float32)
    w["bout"] = np.asarray(params["out_lin"][1], np.float32)
    betas = {}
    for li, lp in enumerate(params["layers"]):
        for nt in ("paper", "author"):
            sfx = f"_{nt[0]}_{li}"
            w["Wq" + sfx] = np.asarray(lp["q"][nt][0], np.float32)
            w["bq" + sfx] = np.asarray(lp["q"][nt][1], np.float32)
            w["Wa" + sfx] = np.asarray(lp["a"][nt][0], np.float32)
            w["ba" + sfx] = np.asarray(lp["a"][nt][1], np.float32)
            betas["beta" + sfx] = float(1.0 / (1.0 + np.exp(-float(lp["skip"][nt]))))
        for (s, r, d) in EDGE_TYPES:
            Wkt, bkt, Wvt, bvt = _fuse_rel_weights(lp, r, s)
            w[f"Wk_{r}_{li}"], w[f"bk_{r}_{li}"] = Wkt, bkt
            w[f"Wv_{r}_{li}"], w[f"bv_{r}_{li}"] = Wvt, bvt
    return percore, dict(rel_info=rel_info, weights=w, betas=betas)


# ------------------------------------------------------------ bass program
def _build_program(meta):
    import concourse.bacc as bacc
    import concourse.bass as bass
    import concourse.tile as tile
    import concourse.mybir as mybir
    from concourse.masks import make_identity

    dt = mybir.dt
    AF = mybir.ActivationFunctionType
    ALU = mybir.AluOpType
    ds = bass.ds

    w = meta["weights"]
    betas = meta["betas"]
    rel_info = meta["rel_info"]

    nc = bacc.Bacc("TRN2", target_bir_lowering=False, debug=False, num_devices=NC)

    xin_p = nc.dram_tensor("xin_p", [PSL, DIN], dt.float32, kind="ExternalInput")
    xin_a = nc.dram_tensor("xin_a", [ASL, DIN], dt.float32, kind="ExternalInput")
    eidx, lane = {}, {}
    for ri in rel_info:
        r, nt_ = ri["r"], ri["n_tiles"]
        eidx[r] = nc.dram_tensor(f"eidx_{r}", [P, nt_], dt.int32, kind="ExternalInput")
        lane[r] = nc.dram_tensor(f"lane_{r}", [P, nt_], dt.float32, kind="ExternalInput")
    wt = {}
    for k, v in w.items():
        arr = np.atleast_2d(np.asarray(v, np.float32))
        wt[k] = nc.dram_tensor("w_" + k, list(arr.shape), dt.float32, kind="ExternalInput")
    out = nc.dram_tensor("out", [PSL, OUT], dt.float32, kind="ExternalOutput")

    x_all = nc.dram_tensor("x_all", [NC * SL, HID], dt.float32)
    x_stage = nc.dram_tensor("x_stage", [SL, HID], dt.float32)
    x_slice = nc.dram_tensor("x_slice", [SL + P, HID], dt.float32)
    qtab = nc.dram_tensor("qtab", [SL + 4096, HID], dt.float32)
    U = {}
    for ri in rel_info:
        U[ri["r"]] = nc.dram_tensor(f"U_{ri['r']}",
                                    [ri["S"] * ri["n_tiles"] + P, 136], dt.float32)

    from contextlib import ExitStack
    with tile.TileContext(nc) as tc, ExitStack() as stk:
        wpool = stk.enter_context(tc.tile_pool(name="wts", bufs=1))
        pool = stk.enter_context(tc.tile_pool(name="work", bufs=3))
        ppool = stk.enter_context(tc.tile_pool(name="psum", bufs=2, space="PSUM"))

        ident = wpool.tile([P, P], dt.float32, name="ident")
        make_identity(nc, ident[:])
        iota_i = wpool.tile([P, P], dt.int32, name="iota_i")
        nc.gpsimd.iota(iota_i[:], pattern=[[1, P]], base=0, channel_multiplier=0)
        iota_row = wpool.tile([P, P], dt.float32, name="iota_row")
        nc.vector.tensor_copy(out=iota_row[:], in_=iota_i[:])
        ones1 = wpool.tile([1, P], dt.float32, name="ones1")
        nc.vector.memset(ones1[:], 1.0)

        wtile = {}
        for k, t in wt.items():
            arr = np.atleast_2d(np.asarray(w[k], np.float32))
            wtile[k] = wpool.tile(list(arr.shape), dt.float32, name="wt_" + k)
            nc.sync.dma_start(out=wtile[k][:], in_=t[:, :])

        def barrier():
            tc.strict_bb_all_engine_barrier()

        # ---- generic linear block: dst = f(src @ W + b) [+ skip mix]
        def lin_block(dst_ap, src_ap, Wt, bt, act=None, beta=None, skip_ap=None,
                      nrows=P, ncols=HID):
            xt = pool.tile([P, P], dt.float32, tag="lb_x", name="lb_x")
            nc.sync.dma_start(out=xt[:nrows, :], in_=src_ap)
            xT_ps = ppool.tile([P, 2 * P], dt.float32, tag="ps_a", name="ps_a")
            nc.tensor.transpose(out=xT_ps[:, :P], in_=xt[:], identity=ident[:])
            xT = pool.tile([P, P], dt.float32, tag="lb_xTs", name="lb_xTs")
            nc.vector.tensor_copy(out=xT[:], in_=xT_ps[:, :P])
            o_ps = ppool.tile([P, 2 * P], dt.float32, tag="ps_b", name="ps_b")
            nc.tensor.matmul(out=o_ps[:, :ncols], lhsT=xT[:], rhs=Wt[:, :ncols],
                             start=True, stop=False)
            nc.tensor.matmul(out=o_ps[:, :ncols], lhsT=ones1[:],
                             rhs=bt[:1, :ncols], start=False, stop=True)
            o = pool.tile([P, P], dt.float32, tag="lb_os", name="lb_os")
            if act is not None:
                nc.scalar.activation(out=o[:, :ncols], in_=o_ps[:, :ncols], func=act)
            elif beta is not None:
                nc.scalar.mul(out=o[:, :ncols], in_=o_ps[:, :ncols], mul=float(beta))
                sk = pool.tile([P, P], dt.float32, tag="lb_sk", name="lb_sk")
                nc.scalar.mul(out=sk[:nrows, :], in_=skip_ap, mul=float(1.0 - beta))
                nc.vector.tensor_add(out=o[:nrows, :ncols], in0=o[:nrows, :ncols],
                                     in1=sk[:nrows, :ncols])
            else:
                nc.scalar.copy(out=o[:, :ncols], in_=o_ps[:, :ncols])
            nc.sync.dma_start(out=dst_ap, in_=o[:nrows, :ncols])

        # ---- run fn over n blocks of P rows: For_i over full UN groups,
        #      then static remainder (fn receives a row-offset expression).
        def loop_blocks(n_blocks, fn):
            ngrp = n_blocks // UN
            if ngrp > 0:
                with tc.For_i(0, ngrp, 1) as g:
                    for j in range(UN):
                        fn(g * UN + j, static=False)
            for m in range(ngrp * UN, n_blocks):
                fn(m, static=True)

        # ============ phase 0: in_lin -> x_stage ; copy to x_slice
        def in_p(m, static):
            lin_block(x_stage[ds(m * P, P), :], xin_p[ds(m * P, P), :],
                      wtile["Win_p"], wtile["bin_p"], act=AF.Relu)
        loop_blocks(PSL // P, in_p)          # 97 full paper blocks
        lin_block(x_stage[ds(PSL - 84, 84), :], xin_p[ds(PSL - 84, 84), :],
                  wtile["Win_p"], wtile["bin_p"], act=AF.Relu, nrows=84)

        def in_a(m, static):
            lin_block(x_stage[ds(PSL + m * P, P), :], xin_a[ds(m * P, P), :],
                      wtile["Win_a"], wtile["bin_a"], act=AF.Relu)
        loop_blocks(ASL // P, in_a)          # 48 full author blocks
        lin_block(x_stage[ds(SL - 106, 106), :], xin_a[ds(ASL - 106, 106), :],
                  wtile["Win_a"], wtile["bin_a"], act=AF.Relu, nrows=106)
        barrier()

        def copy_stage_to_slice():
            def cp(m, static):
                c = pool.tile([P, HID], dt.float32, tag="xcopy", name="xcopy")
                nc.sync.dma_start(out=c[:], in_=x_stage[ds(m * P, P), :])
                nc.sync.dma_start(out=x_slice[ds(m * P, P), :], in_=c[:])
            loop_blocks(SL // P, cp)         # 146 full
            c = pool.tile([P, HID], dt.float32, tag="xcopy", name="xcopy")
            nc.sync.dma_start(out=c[:62, :], in_=x_stage[ds(SL - 62, 62), :])
            nc.sync.dma_start(out=x_slice[ds(SL - 62, 62), :], in_=c[:62, :])

        copy_stage_to_slice()
        barrier()

        # ============ layers
        for li in range(LAYERS):
            nc.gpsimd.collective_compute(
                "AllGather", ALU.bypass, replica_groups=[list(range(NC))],
                ins=[x_stage.ap().opt()], outs=[x_all.ap().opt()])
            barrier()

            # ---- q build
            def q_p(m, static):
                lin_block(qtab[ds(m * P, P), :], x_slice[ds(m * P, P), :],
                          wtile[f"Wq_p_{li}"], wtile[f"bq_p_{li}"])
            loop_blocks(PSL // P, q_p)
            lin_block(qtab[ds(PSL - 84, 84), :], x_slice[ds(PSL - 84, 84), :],
                      wtile[f"Wq_p_{li}"], wtile[f"bq_p_{li}"], nrows=84)

            def q_a(m, static):
                lin_block(qtab[ds(PSL + m * P, P), :], x_slice[ds(PSL + m * P, P), :],
                          wtile[f"Wq_a_{li}"], wtile[f"bq_a_{li}"])
            loop_blocks(ASL // P, q_a)
            lin_block(qtab[ds(SL - 106, 106), :], x_slice[ds(SL - 106, 106), :],
                      wtile[f"Wq_a_{li}"], wtile[f"bq_a_{li}"], nrows=106)
            barrier()

            # ---- edge loops
            for ri in rel_info:
                r, S, NT = ri["r"], ri["S"], ri["n_tiles"]
                qoff = 0 if ri["d"] == "paper" else PSL
                Wk, bk = wtile[f"Wk_{r}_{li}"], wtile[f"bk_{r}_{li}"]
                Wv, bv = wtile[f"Wv_{r}_{li}"], wtile[f"bv_{r}_{li}"]
                Ur = U[r]

                def tile_body(tt):
                    idx_t = pool.tile([P, 1], dt.int32, tag="e_idx", name="e_idx")
                    nc.sync.dma_start(out=idx_t[:], in_=eidx[r][:, ds(tt, 1)])
                    ln_t = pool.tile([P, 1], dt.float32, tag="e_ln", name="e_ln")
                    nc.sync.dma_start(out=ln_t[:], in_=lane[r][:, ds(tt, 1)])
                    xg = pool.tile([P, P], dt.float32, tag="e_xg", name="e_xg")
                    nc.gpsimd.indirect_dma_start(
                        out=xg[:], out_offset=None, in_=x_all[:, :],
                        in_offset=bass.IndirectOffsetOnAxis(ap=idx_t[:, :1], axis=0))
                    qr = pool.tile([P, P], dt.float32, tag="e_qr", name="e_qr")
                    nc.sync.dma_start(out=qr[:], in_=qtab[ds(qoff + tt * S, P), :])
                    oh = pool.tile([P, P], dt.float32, tag="e_oh", name="e_oh")
                    nc.vector.tensor_tensor(out=oh[:],
                                            in0=ln_t[:, :1].to_broadcast([P, P]),
                                            in1=iota_row[:], op=ALU.is_equal)
                    tp_ps = ppool.tile([P, 2 * P], dt.float32, tag="ps_a", name="ps_a")
                    nc.tensor.transpose(out=tp_ps[:, :P], in_=xg[:], identity=ident[:])
                    nc.tensor.transpose(out=tp_ps[:, P:], in_=oh[:], identity=ident[:])
                    tp = pool.tile([P, 2 * P], dt.float32, tag="e_tps", name="e_tps")
                    nc.vector.tensor_copy(out=tp[:], in_=tp_ps[:])
                    xT, ohT = tp[:, :P], tp[:, P:]
                    kv_ps = ppool.tile([P, 2 * P], dt.float32, tag="ps_b", name="ps_b")
                    nc.tensor.matmul(out=kv_ps[:, :P], lhsT=xT, rhs=Wk[:],
                                     start=True, stop=False)
                    nc.tensor.matmul(out=kv_ps[:, :P], lhsT=ones1[:], rhs=bk[:1, :],
                                     start=False, stop=True)
                    nc.tensor.matmul(out=kv_ps[:, P:], lhsT=xT, rhs=Wv[:],
                                     start=True, stop=False)
                    nc.tensor.matmul(out=kv_ps[:, P:], lhsT=ones1[:], rhs=bv[:1, :],
                                     start=False, stop=True)
                    qd_ps = ppool.tile([P, 2 * P], dt.float32, tag="ps_c", name="ps_c")
                    nc.tensor.matmul(out=qd_ps[:, :P], lhsT=ohT, rhs=qr[:],
                                     start=True, stop=True)
                    qd = pool.tile([P, P], dt.float32, tag="e_qd", name="e_qd")
                    nc.scalar.copy(out=qd[:], in_=qd_ps[:, :P])
                    pr = pool.tile([P, P], dt.float32, tag="e_pr", name="e_pr")
                    nc.vector.tensor_mul(out=pr[:], in0=kv_ps[:, :P], in1=qd[:])
                    sc = pool.tile([P, 8], dt.float32, tag="e_sc", name="e_sc")
                    nc.vector.reduce_sum(out=sc[:, :H],
                                         in_=pr[:].rearrange("p (h d) -> p h d", h=H),
                                         axis=mybir.AxisListType.X)
                    esc = pool.tile([P, 8], dt.float32, tag="e_es", name="e_es")
                    nc.scalar.activation(out=esc[:, :H], in_=sc[:, :H], func=AF.Exp)
                    rhs = pool.tile([P, 136], dt.float32, tag="e_rh", name="e_rh")
                    nc.vector.tensor_mul(
                        out=rhs[:, :HID].rearrange("p (h d) -> p h d", h=H),
                        in0=kv_ps[:, P:].rearrange("p (h d) -> p h d", h=H),
                        in1=esc[:, :H].to_broadcast([P, H, D]))
                    nc.vector.tensor_copy(out=rhs[:, HID:HID + H], in_=esc[:, :H])
                    nc.vector.memset(rhs[:, HID + H:], 0.0)
                    u_ps = ppool.tile([P, 2 * P], dt.float32, tag="ps_d", name="ps_d")
                    nc.tensor.matmul(out=u_ps[:, :136], lhsT=oh[:], rhs=rhs[:],
                                     start=True, stop=True)
                    u_sb = pool.tile([P, 136], dt.float32, tag="e_ub", name="e_ub")
                    nc.scalar.copy(out=u_sb[:], in_=u_ps[:, :136])
                    nc.sync.dma_start(out=Ur[ds(tt * S, S), :], in_=u_sb[:S, :])

                with tc.For_i(0, NT // UN, 1) as it:
                    for j in range(UN):
                        tile_body(it * UN + j)
            barrier()

            # ---- epilogue (uoff = row offset in the dst-type-local U tables)
            def epi_block(nt_, uoff, nrows):
                if nt_ == "p":
                    rels = ["cites", "writes"]
                    xoff = 0
                    Wa, ba = wtile[f"Wa_p_{li}"], wtile[f"ba_p_{li}"]
                    beta = betas[f"beta_p_{li}"]
                else:
                    rels = ["rev_writes"]
                    xoff = PSL
                    Wa, ba = wtile[f"Wa_a_{li}"], wtile[f"ba_a_{li}"]
                    beta = betas[f"beta_a_{li}"]
                u0 = pool.tile([P, 136], dt.float32, tag="ep_u0", name="ep_u0")
                nc.sync.dma_start(out=u0[:nrows, :], in_=U[rels[0]][ds(uoff, nrows), :])
                if len(rels) > 1:
                    u1 = pool.tile([P, 136], dt.float32, tag="ep_u1", name="ep_u1")
                    nc.sync.dma_start(out=u1[:nrows, :],
                                      in_=U[rels[1]][ds(uoff, nrows), :])
                    nc.vector.tensor_add(out=u0[:nrows, :], in0=u0[:nrows, :],
                                         in1=u1[:nrows, :])
                den = pool.tile([P, 8], dt.float32, tag="ep_den", name="ep_den")
                nc.vector.tensor_scalar_max(den[:, :H], u0[:, HID:HID + H], 1e-30)
                rec = pool.tile([P, 8], dt.float32, tag="ep_rec", name="ep_rec")
                nc.vector.reciprocal(out=rec[:, :H], in_=den[:, :H])
                agg = pool.tile([P, P], dt.float32, tag="ep_agg", name="ep_agg")
                nc.vector.tensor_mul(
                    out=agg[:].rearrange("p (h d) -> p h d", h=H),
                    in0=u0[:, :HID].rearrange("p (h d) -> p h d", h=H),
                    in1=rec[:, :H].to_broadcast([P, H, D]))
                g = pool.tile([P, P], dt.float32, tag="ep_g", name="ep_g")
                nc.scalar.activation(out=g[:], in_=agg[:], func=AF.Gelu)
                gT_ps = ppool.tile([P, 2 * P], dt.float32, tag="ps_a", name="ps_a")
                nc.tensor.transpose(out=gT_ps[:, :P], in_=g[:], identity=ident[:])
                gT = pool.tile([P, P], dt.float32, tag="ep_gTs", name="ep_gTs")
                nc.vector.tensor_copy(out=gT[:], in_=gT_ps[:, :P])
                o_ps = ppool.tile([P, 2 * P], dt.float32, tag="ps_b", name="ps_b")
                nc.tensor.matmul(out=o_ps[:, :P], lhsT=gT[:], rhs=Wa[:],
                                 start=True, stop=False)
                nc.tensor.matmul(out=o_ps[:, :P], lhsT=ones1[:], rhs=ba[:1, :],
                                 start=False, stop=True)
                o = pool.tile([P, P], dt.float32, tag="ep_os", name="ep_os")
                nc.scalar.mul(out=o[:], in_=o_ps[:, :P], mul=beta)
                xsk = pool.tile([P, P], dt.float32, tag="ep_xsk", name="ep_xsk")
                nc.sync.dma_start(out=xsk[:nrows, :],
                                  in_=x_slice[ds(xoff + uoff, nrows), :])
                sk = pool.tile([P, P], dt.float32, tag="ep_sk", name="ep_sk")
                nc.scalar.mul(out=sk[:nrows, :], in_=xsk[:nrows, :], mul=1.0 - beta)
                nc.vector.tensor_add(out=o[:nrows, :], in0=o[:nrows, :],
                                     in1=sk[:nrows, :])
                nc.sync.dma_start(out=x_stage[ds(xoff + uoff, nrows), :],
                                  in_=o[:nrows, :])

            loop_blocks(PSL // P, lambda m, static: epi_block("p", m * P, P))
            epi_block("p", PSL - 84, 84)
            loop_blocks(ASL // P, lambda m, static: epi_block("a", m * P, P))
            epi_block("a", ASL - 106, 106)
            barrier()

            if li + 1 < LAYERS:
                copy_stage_to_slice()
                barrier()

        # ============ out_lin (papers)
        def out_b(m, static):
            lin_block(out[ds(m * P, P), :], x_stage[ds(m * P, P), :],
                      wtile["Wout"], wtile["bout"], ncols=OUT)
        loop_blocks(PSL // P, out_b)
        lin_block(out[ds(PSL - 84, 84), :], x_stage[ds(PSL - 84, 84), :],
                  wtile["Wout"], wtile["bout"], ncols=OUT, nrows=84)

    nc.compile()
    return nc


class _Runner:
    """Persistent PJRT executor for the compiled Bass program (axon path)."""

    def __init__(self, nc, n_cores):
        import jax
        from jax.sharding import Mesh, PartitionSpec
        from jax.experimental.shard_map import shard_map
        import concourse.mybir as mybir
        from concourse.bass2jax import (_bass_exec_p, install_neuronx_cc_hook,
                                        partition_id_tensor)

        install_neuronx_cc_hook()
        self.jax = jax
        self.n = n_cores
        in_names, out_names, out_avals, zero_outs = [], [], [], []
        for alloc in nc.m.functions[0].allocations:
            if not isinstance(alloc, mybir.MemoryLocationSet):
                continue
            name = alloc.memorylocations[0].name
            if alloc.kind == "ExternalInput":
                in_names.append(name)
            elif alloc.kind == "ExternalOutput":
                shape = tuple(alloc.tensor_shape)
                dtype = mybir.dt.np(alloc.dtype)
                out_names.append(name)
                out_avals.append(jax.core.ShapedArray(shape, dtype))
                zero_outs.append(np.zeros(shape, dtype))
        pname = nc.partition_id_tensor.name if nc.partition_id_tensor else None
        if pname is not None:
            in_names = [n for n in in_names if n != pname]
        self.in_names, self.out_names = in_names, out_names
        self.out_avals, self.zero_outs = out_avals, zero_outs
        n_params, n_outs = len(in_names), len(out_avals)
        all_in = tuple(in_names + out_names + ([pname] if pname else []))

        def _body(*args):
            operands = list(args)
            if pname is not None:
                operands.append(partition_id_tensor())
            return tuple(_bass_exec_p.bind(
                *operands, out_avals=tuple(out_avals), in_names=all_in,
                out_names=tuple(out_names), lowering_input_output_aliases=(),
                sim_require_finite=True, sim_require_nnan=True, nc=nc))

        devices = jax.devices()[:n_cores]
        self.mesh = Mesh(np.asarray(devices), ("core",))
        specs = (PartitionSpec("core"),) * (n_params + n_outs)
        self.fn = jax.jit(
            shard_map(_body, mesh=self.mesh, in_specs=specs,
                      out_specs=(PartitionSpec("core"),) * n_outs,
                      check_rep=False),
            keep_unused=True)
        self._PartitionSpec = PartitionSpec

    def stage(self, in_maps):
        jax = self.jax
        sh = jax.sharding.NamedSharding(self.mesh, self._PartitionSpec("core"))
        self._din = [
            jax.device_put(np.concatenate(
                [np.asarray(in_maps[c][k]) for c in range(self.n)], axis=0), sh)
            for k in self.in_names]
        self._dz = [
            jax.device_put(np.zeros((self.n * z.shape[0], *z.shape[1:]), z.dtype), sh)
            for z in self.zero_outs]
        jax.block_until_ready(self._din)
        jax.block_until_ready(self._dz)

    def run(self):
        outs = self.fn(*self._din, *self._dz)
        self.jax.block_until_ready(outs)
        return outs

    def results(self, outs):
        return [
            {k: np.asarray(outs[i]).reshape(self.n, *self.out_avals[i].shape)[c]
             for i, k in enumerate(self.out_names)}
            for c in range(self.n)]

    def time_runs(self, iters=10, warmup=2):
        import time
        for _ in range(warmup):
            outs = self.run()
        ts = []
        for _ in range(iters):
            t0 = time.perf_counter()
            outs = self.run()
            ts.append(time.perf_counter() - t0)
        ts = np.array(ts)
        return outs, {"min_s": float(ts.min()), "median_s": float(np.median(ts)),
                      "mean_s": float(ts.mean())}


_CACHE = {}
_LAST_RUNNER = None


def kernel(**inputs):
    percore, meta = _prep(inputs)
    if "prog" not in _CACHE:
        _CACHE["prog"] = _build_program(meta)
    nc = _CACHE["prog"]

    w = meta["weights"]
    in_maps = []
    for c in range(NC):
        m = {"xin_p": percore[c]["xin_p"], "xin_a": percore[c]["xin_a"]}
        for ri in meta["rel_info"]:
            r = ri["r"]
            m[f"eidx_{r}"] = percore[c][f"eidx_{r}"]
            m[f"lane_{r}"] = percore[c][f"lane_{r}"]
        for k, v in w.items():
            m["w_" + k] = np.ascontiguousarray(np.atleast_2d(np.asarray(v, np.float32)))
        in_maps.append(m)

    global _LAST_RUNNER
    if "runner" not in _CACHE:
        _CACHE["runner"] = _Runner(nc, NC)
    runner = _CACHE["runner"]
    _LAST_RUNNER = runner
    runner.stage(in_maps)
    outs = runner.run()
    res = runner.results(outs)
    return np.concatenate([res[c]["out"] for c in range(NC)], axis=0)
